# revision 2
# baseline (speedup 1.0000x reference)
"""Trainium2 Bass kernel for nn_ConvolutionLayer (5x5 VALID conv).

Full inputs:  x (16,32,224,224) f32, weight (64,32,5,5) f32, bias (64,) f32
Full output:  (16,64,220,220) f32

Sharding: data-parallel over batch — 2 images per core on 8 cores.

Per-core algorithm (bf16 matmuls, fp32 PSUM accumulate):
  - x in SBUF as row-quad blocks [128=(kh,c), 456=(img,w)+pad] on two
    grids (G0: rows 4b.., G1: rows 4b+2..). For output row h the 4-row
    "main" window (K=128) comes from one grid block; the 5th row is
    handled by ONE fused strip matmul -> 4 matmuls/row total:
      m1 {kw0,kw2}@sh0, m2 {kw1,kw3}@sh1, m3 {kw4}@sh4 (M=64),
      m4 strip: replica tile R[128=(4c+s), 452] = strip row at 4 column
      shifts; Ws[(4c+s), f] = w[f,c,kh_s,s] (lo, offset 0) and
      Ws[(4c+2), 64+f] = w[f,c,kh_s,4] (hi, offset +2, matching the
      mains' hi half). All four accumulate into ONE psum bank P[128,448]:
        out[f,h,w] = P[f, 224n+w] + P[64+f, 224n+w+2] + bias[f].
  - Replica tiles are host-prepped in HBM (xg2, one [128, 4*452] group
    per row-quad block) and loaded 1 DMA/block.
  - Epilogue per row (PSUM can be read by ONE operand per instr, and
    only by ACT/DVE):
      op1 (ACT): stage = Identity(P_lo) + bias      [64, 2, 220] APs
      op2 (DVE): stage += P_hi(+2)                  [64, 2, 220] APs
    Stage [64=f, (n, hh, w)] bf16; DMA'd to HBM every GH rows with an
    n-outer SBUF AP -> "(n f) (h w)" HBM AP. Host converts to fp32.
  - Queues: ACT: g0 loads + op1. SP: g1/xg2/weights/out. DVE: op2.
"""
import sys

sys.path.insert(0, "/opt/trn_rl_repo")

import numpy as np
import ml_dtypes
import concourse.bacc as bacc
import concourse.mybir as mybir
from concourse.tile import TileContext
from concourse.bass_utils import run_bass_kernel_spmd
from concourse.ap import AP

F32 = mybir.dt.float32
BF16 = mybir.dt.bfloat16

N_CORES = 8
B, C, H, W = 16, 32, 224, 224
F, K = 64, 5
HO, WO = H - K + 1, W - K + 1  # 220, 220
NB = 2                      # images per core
NBLK = H // 4               # 56 row-quad blocks per grid
NW = NB * W                 # 448 data columns per block
NWP = NW + 8                # padded block width
SW = NW + 4                 # strip tile width
GH = 10                     # output rows per staging/DMA group

_cache = {}


def _build(reps=1, xbufs=4, psbufs=6, stbufs=2, spbufs=3):
    nc = bacc.Bacc(trn_type="TRN2")

    xg0 = nc.dram_tensor("xg0", [NBLK, 128, NWP], BF16, kind="ExternalInput")
    xg1 = nc.dram_tensor("xg1", [NBLK - 1, 128, NWP], BF16,
                         kind="ExternalInput")
    xg2 = nc.dram_tensor("xg2", [55, 128, 4 * SW], BF16, kind="ExternalInput")
    w02 = nc.dram_tensor("w02", [2, 128, 128], BF16, kind="ExternalInput")
    w13 = nc.dram_tensor("w13", [2, 128, 128], BF16, kind="ExternalInput")
    w4 = nc.dram_tensor("w4", [2, 128, 64], BF16, kind="ExternalInput")
    ws = nc.dram_tensor("ws", [2, 128, 128], BF16, kind="ExternalInput")
    bias = nc.dram_tensor("bias", [64, 1], F32, kind="ExternalInput")
    out = nc.dram_tensor("out", [NB, F, HO, WO], BF16, kind="ExternalOutput")

    with TileContext(nc) as tc:
        with (
            tc.tile_pool(name="wp", bufs=1) as wp,
            tc.tile_pool(name="bp", bufs=1) as bp,
            tc.tile_pool(name="xp", bufs=xbufs) as xp,
            tc.tile_pool(name="sp", bufs=spbufs) as sp,
            tc.tile_pool(name="pp", bufs=psbufs, space="PSUM") as pp,
            tc.tile_pool(name="op", bufs=stbufs) as op,
        ):
            wt = {}
            for name, wd, m in (("02", w02, 128), ("13", w13, 128),
                                ("4", w4, 64)):
                t1 = wp.tile([128, m], BF16, tag=f"t1{name}")
                t2 = wp.tile([128, m], BF16, tag=f"t2{name}")
                nc.sync.dma_start(out=t1[:], in_=wd[0, :, :])
                nc.sync.dma_start(out=t2[:], in_=wd[1, :, :])
                wt[name] = (t1, t2)
            ws0 = wp.tile([128, 128], BF16, tag="ws0")
            ws4 = wp.tile([128, 128], BF16, tag="ws4")
            nc.sync.dma_start(out=ws0[:], in_=ws[0, :, :])
            nc.sync.dma_start(out=ws4[:], in_=ws[1, :, :])
            bt = bp.tile([64, 1], F32)
            nc.sync.dma_start(out=bt[:], in_=bias[:])

            g0_tiles, g1_tiles, g2_tiles = {}, {}, {}

            def load_g0(b):
                t = xp.tile([128, NWP], BF16, tag="x0")
                nc.scalar.dma_start(out=t[:], in_=xg0[b, :, :])
                g0_tiles[b] = t

            def load_g1(b):
                t = xp.tile([128, NWP], BF16, tag="x1")
                nc.sync.dma_start(out=t[:], in_=xg1[b, :, :])
                g1_tiles[b] = t

            def load_g2(b):
                t = sp.tile([128, 4 * SW], BF16, tag="st")
                nc.sync.dma_start(out=t[:], in_=xg2[b, :, :])
                g2_tiles[b] = t

            def emit_pass():
                g0_tiles.clear()
                g1_tiles.clear()
                g2_tiles.clear()
                load_g0(0)
                load_g1(0)
                load_g2(0)
                stage_map = {}

                def get_stage(h):
                    g = h // GH
                    if g not in stage_map:
                        stage = op.tile([64, NB * GH * WO], BF16,
                                        tag="stage")
                        stage_map[g] = stage
                    return stage_map[g]
                for b in range(55):
                    load_g0(b + 1)
                    if b + 1 <= 54:
                        load_g1(b + 1)
                        load_g2(b + 1)
                    # phases: rows (0,2) share T1-set + ws4; (1,3) share
                    # T2-set + ws0 — interleave so each lhsT feeds two
                    # consecutive matmuls (amortizes PE weight loads).
                    for ra, rb in ((0, 2), (1, 3)):
                        mains = {
                            0: g0_tiles[b], 1: g1_tiles[b],
                            2: g1_tiles[b], 3: g0_tiles[b + 1],
                        }
                        wstrip = ws4 if ra == 0 else ws0
                        wi = 0 if ra == 0 else 1
                        pss, stages, hhs = {}, {}, {}
                        for r in (ra, rb):
                            h = 4 * b + r
                            stages[r] = get_stage(h)
                            hhs[r] = h % GH
                            pss[r] = pp.tile([128, NW], F32, tag="ps", name="ps")
                        for gname, m, sh in (("02", 128, 0), ("13", 128, 1),
                                             ("4", 64, 4)):
                            wm = wt[gname][wi]
                            for r in (ra, rb):
                                nc.tensor.matmul(
                                    out=pss[r][0:m, 0:NW],
                                    lhsT=wm[0:128, 0:m],
                                    rhs=mains[r][0:128, sh:sh + NW],
                                    start=(gname == "02"), stop=False)
                        for r in (ra, rb):
                            nc.tensor.matmul(
                                out=pss[r][0:128, 0:NW],
                                lhsT=wstrip[0:128, 0:128],
                                rhs=g2_tiles[b][0:128, r * SW:r * SW + NW],
                                start=False, stop=True)
                        for r in (ra, rb):
                            h = 4 * b + r
                            hh = hhs[r]
                            ps_t = pss[r][:].tensor
                            st_t = stages[r][:].tensor
                            ps_lo = AP(ps_t, 0, [[NW, 64], [W, 2], [1, WO]])
                            ps_hi = AP(ps_t, 64 * NW + 2,
                                       [[NW, 64], [W, 2], [1, WO]])
                            st_ap = AP(st_t, hh * WO,
                                       [[NB * GH * WO, 64], [GH * WO, 2],
                                        [1, WO]])
                            nc.scalar.activation(
                                out=st_ap, in_=ps_lo,
                                func=mybir.ActivationFunctionType.Identity,
                                bias=bt[:], scale=1.0)
                            nc.vector.tensor_tensor(
                                out=st_ap, in0=ps_hi, in1=st_ap,
                                op=mybir.AluOpType.add)
                            if hh == GH - 1:
                                h0 = h - GH + 1
                                nc.gpsimd.dma_start(
                                    out=out[:, :, h0:h0 + GH, :].transpose(
                                        [1, 0, 2, 3]),
                                    in_=stages[r][:],
                                )

            for _ in range(reps):
                emit_pass()

    nc.finalize()
    return nc


def _prep_core(xs, weight, bias):
    """xs: (2,32,224,224) f32 -> per-core input map (bf16 grids/weights)."""
    xb = xs.astype(ml_dtypes.bfloat16)

    def _grid(arr, nblk):
        g = arr.reshape(NB, C, -1, 4, W)[:, :, :nblk].transpose(2, 3, 1, 0, 4)
        o = np.zeros((nblk, 128, NWP), ml_dtypes.bfloat16)
        o[:, :, :NW] = g.reshape(nblk, 128, NW)
        return o

    g0 = _grid(xb, NBLK)
    xpad = np.concatenate(
        [xb[:, :, 2:, :],
         np.zeros((NB, C, 2, W), ml_dtypes.bfloat16)], axis=2)
    g1 = _grid(xpad, NBLK - 1)

    # xg2[b][4c+s, 452*r + col] = xrowpad(strip_row(4b+r))[c, col+s]
    # strip_row(h) = h+4 if h even else h
    rowbuf = np.zeros((C, H, NWP), ml_dtypes.bfloat16)
    rowbuf[:, :, :NW] = xb.transpose(1, 2, 0, 3).reshape(C, H, NW)
    g2 = np.zeros((55, 128, 4 * SW), ml_dtypes.bfloat16)
    for r in range(4):
        hs = np.arange(r, 220, 4)           # output rows with this r
        srows = np.where(hs % 2 == 0, hs + 4, hs)   # strip rows
        for s in range(4):
            # g2[b, 4c+s, 452r + col] = rowbuf[c, srows[b], col+s]
            g2[:, s::4, r * SW:(r + 1) * SW] = \
                rowbuf[:, srows, s:s + SW].transpose(1, 0, 2)
    wb = weight.astype(ml_dtypes.bfloat16).astype(np.float32)

    def _main(kws):
        o = np.zeros((2, 128, len(kws) * 64), np.float32)
        for t in range(2):
            for j in range(4):
                for mi, kw in enumerate(kws):
                    o[t, 32 * j:32 * j + 32, 64 * mi:64 * mi + 64] = \
                        wb[:, :, j + t, kw].T
        return o.astype(ml_dtypes.bfloat16)

    wsm = np.zeros((2, 128, 128), np.float32)
    for i, kh in enumerate((0, 4)):
        for s in range(4):
            wsm[i, s::4, 0:64] = wb[:, :, kh, s].T
        wsm[i, 2::4, 64:128] = wb[:, :, kh, 4].T

    return {
        "xg0": g0,
        "xg1": g1,
        "xg2": g2,
        "w02": _main((0, 2)),
        "w13": _main((1, 3)),
        "w4": _main((4,)),
        "ws": wsm.astype(ml_dtypes.bfloat16),
        "bias": bias.reshape(64, 1).astype(np.float32),
    }


def kernel(x, weight, bias, _profile=False):
    x = np.asarray(x, dtype=np.float32)
    weight = np.asarray(weight, dtype=np.float32)
    bias = np.asarray(bias, dtype=np.float32)

    if "nc" not in _cache:
        _cache["nc"] = _build()
    nc = _cache["nc"]

    in_maps = [
        _prep_core(x[NB * i:NB * i + NB], weight, bias) for i in range(N_CORES)
    ]
    res = run_bass_kernel_spmd(
        nc, in_maps, core_ids=list(range(N_CORES)), trace=_profile)
    out = np.concatenate(
        [np.asarray(r["out"]).astype(np.float32) for r in res.results], axis=0)
    if _profile:
        _cache["last_results"] = res
    return out


if __name__ == "__main__":
    rng = np.random.default_rng(0)
    x = rng.standard_normal((B, C, H, W), dtype=np.float32)
    w = rng.standard_normal((F, C, K, K), dtype=np.float32)
    bv = rng.standard_normal((F,), dtype=np.float32)
    o = kernel(x, w, bv)
    print("output shape:", o.shape, o.dtype)


# revision 3
# speedup vs baseline: 1.3130x; 1.3130x over previous
"""Trainium2 Bass kernel v3 for nn_ConvolutionLayer (5x5 VALID conv).

Full inputs:  x (16,32,224,224) f32, weight (64,32,5,5) f32, bias (64,) f32
Full output:  (16,64,220,220) f32

Sharding: data-parallel over batch — 2 images per core on 8 cores.

Per-core algorithm (bf16 matmuls, fp32 PSUM accumulate):
  - x in SBUF as row-quad blocks [128=(kh,c), 456=(img,w)+pad] on two
    grids (G0: rows 4b.., G1: rows 4b+2..). For output row h the 4-row
    "main" window (K=128) comes from one grid block; the 5th row is
    handled by ONE fused strip matmul -> 4 matmuls/row total:
      m1 {kw0,kw2}@sh0, m2 {kw1,kw3}@sh1, m3 {kw4}@sh4 (M=64),
      m4 strip: replica tile R[128=(4c+s), 452] = strip row at 4 column
      shifts; Ws[(4c+s), f] = w[f,c,kh_s,s] (lo, offset 0) and
      Ws[(4c+2), 64+f] = w[f,c,kh_s,4] (hi, offset +2, matching the
      mains' hi half). All four accumulate into ONE psum bank P[128,448]:
        out[f,h,w] = P[f, 224n+w] + P[64+f, 224n+w+2] + bias[f].
  - Replica tiles are host-prepped in HBM (xg2, one [128, 4*452] group
    per row-quad block) and loaded 1 DMA/block.
  - Epilogue per row (PSUM can be read by ONE operand per instr, and
    only by ACT/DVE):
      op1 (ACT): stage = Identity(P_lo) + bias      [64, 2, 220] APs
      op2 (DVE): stage += P_hi(+2)                  [64, 2, 220] APs
    Stage [64=f, (n, hh, w)] bf16; DMA'd to HBM every GH rows with an
    n-outer SBUF AP -> "(n f) (h w)" HBM AP. Host converts to fp32.
  - Queues: ACT: g0 loads + op1. SP: g1/xg2/weights/out. DVE: op2.
"""
import sys

sys.path.insert(0, "/opt/trn_rl_repo")

import numpy as np
import ml_dtypes
import concourse.bacc as bacc
import concourse.mybir as mybir
from concourse.tile import TileContext
from concourse.bass_utils import run_bass_kernel_spmd
from concourse.ap import AP

F32 = mybir.dt.float32
BF16 = mybir.dt.bfloat16

N_CORES = 8
B, C, H, W = 16, 32, 224, 224
F, K = 64, 5
HO, WO = H - K + 1, W - K + 1  # 220, 220
NB = 2                      # images per core
NBLK = H // 4               # 56 row-quad blocks per grid
NW = NB * W                 # 448 data columns per block
NWP = NW + 8                # padded block width
SW = NW + 4                 # strip tile width
GH = 10                     # output rows per staging/DMA group

_cache = {}


def _build(reps=1, xbufs=6, psbufs=8, stbufs=3, spbufs=4):
    nc = bacc.Bacc(trn_type="TRN2")

    xg0 = nc.dram_tensor("xg0", [NBLK, 128, NWP], BF16, kind="ExternalInput")
    xg1 = nc.dram_tensor("xg1", [NBLK - 1, 128, NWP], BF16,
                         kind="ExternalInput")
    xg2 = nc.dram_tensor("xg2", [55, 128, 4 * SW], BF16, kind="ExternalInput")
    w02 = nc.dram_tensor("w02", [2, 128, 128], BF16, kind="ExternalInput")
    w13 = nc.dram_tensor("w13", [2, 128, 128], BF16, kind="ExternalInput")
    w4 = nc.dram_tensor("w4", [2, 128, 64], BF16, kind="ExternalInput")
    ws = nc.dram_tensor("ws", [2, 128, 128], BF16, kind="ExternalInput")
    bias = nc.dram_tensor("bias", [64, 1], F32, kind="ExternalInput")
    out = nc.dram_tensor("out", [NB, F, HO, WO], BF16, kind="ExternalOutput")

    with TileContext(nc) as tc:
        with (
            tc.tile_pool(name="wp", bufs=1) as wp,
            tc.tile_pool(name="bp", bufs=1) as bp,
            tc.tile_pool(name="xp", bufs=xbufs) as xp,
            tc.tile_pool(name="sp", bufs=spbufs) as sp,
            tc.tile_pool(name="pp", bufs=psbufs, space="PSUM") as pp,
            tc.tile_pool(name="op", bufs=stbufs) as op,
        ):
            wt = {}
            for name, wd, m in (("02", w02, 128), ("13", w13, 128),
                                ("4", w4, 64)):
                t1 = wp.tile([128, m], BF16, tag=f"t1{name}")
                t2 = wp.tile([128, m], BF16, tag=f"t2{name}")
                nc.sync.dma_start(out=t1[:], in_=wd[0, :, :])
                nc.sync.dma_start(out=t2[:], in_=wd[1, :, :])
                wt[name] = (t1, t2)
            ws0 = wp.tile([128, 128], BF16, tag="ws0")
            ws4 = wp.tile([128, 128], BF16, tag="ws4")
            nc.sync.dma_start(out=ws0[:], in_=ws[0, :, :])
            nc.sync.dma_start(out=ws4[:], in_=ws[1, :, :])
            bt = bp.tile([64, 1], F32)
            nc.sync.dma_start(out=bt[:], in_=bias[:])

            g0_tiles, g1_tiles, g2_tiles = {}, {}, {}

            def load_g0(b):
                t = xp.tile([128, NWP], BF16, tag="x0")
                nc.scalar.dma_start(out=t[:], in_=xg0[b, :, :])
                g0_tiles[b] = t

            def load_g1(b):
                t = xp.tile([128, NWP], BF16, tag="x1")
                nc.sync.dma_start(out=t[:], in_=xg1[b, :, :])
                g1_tiles[b] = t

            def load_g2(b):
                t = sp.tile([128, 4 * SW], BF16, tag="st")
                nc.sync.dma_start(out=t[:], in_=xg2[b, :, :])
                g2_tiles[b] = t

            def emit_pass():
                g0_tiles.clear()
                g1_tiles.clear()
                g2_tiles.clear()
                load_g0(0)
                load_g1(0)
                load_g2(0)
                stage_map = {}

                def get_stage(h):
                    g = h // GH
                    if g not in stage_map:
                        stage = op.tile([64, NB * GH * WO], BF16,
                                        tag="stage")
                        stage_map[g] = stage
                    return stage_map[g]
                for b in range(55):
                    load_g0(b + 1)
                    if b + 1 <= 54:
                        load_g1(b + 1)
                        load_g2(b + 1)
                    # phases: rows (0,2) share T1-set + ws4; (1,3) share
                    # T2-set + ws0 — interleave so each lhsT feeds two
                    # consecutive matmuls (amortizes PE weight loads).
                    for ra, rb in ((0, 2), (1, 3)):
                        mains = {
                            0: g0_tiles[b], 1: g1_tiles[b],
                            2: g1_tiles[b], 3: g0_tiles[b + 1],
                        }
                        wstrip = ws4 if ra == 0 else ws0
                        wi = 0 if ra == 0 else 1
                        pss, stages, hhs = {}, {}, {}
                        for r in (ra, rb):
                            h = 4 * b + r
                            stages[r] = get_stage(h)
                            hhs[r] = h % GH
                            pss[r] = pp.tile([128, NW], F32, tag="ps", name="ps")
                        for gname, m, sh in (("02", 128, 0), ("13", 128, 1),
                                             ("4", 64, 4)):
                            wm = wt[gname][wi]
                            for r in (ra, rb):
                                nc.tensor.matmul(
                                    out=pss[r][0:m, 0:NW],
                                    lhsT=wm[0:128, 0:m],
                                    rhs=mains[r][0:128, sh:sh + NW],
                                    start=(gname == "02"), stop=False)
                        for r in (ra, rb):
                            nc.tensor.matmul(
                                out=pss[r][0:128, 0:NW],
                                lhsT=wstrip[0:128, 0:128],
                                rhs=g2_tiles[b][0:128, r * SW:r * SW + NW],
                                start=False, stop=True)
                        for r in (ra, rb):
                            h = 4 * b + r
                            hh = hhs[r]
                            ps_t = pss[r][:].tensor
                            st_t = stages[r][:].tensor
                            ps_lo = AP(ps_t, 0, [[NW, 64], [W, 2], [1, WO]])
                            ps_hi = AP(ps_t, 64 * NW + 2,
                                       [[NW, 64], [W, 2], [1, WO]])
                            st_ap = AP(st_t, hh * WO,
                                       [[NB * GH * WO, 64], [GH * WO, 2],
                                        [1, WO]])
                            nc.scalar.activation(
                                out=st_ap, in_=ps_lo,
                                func=mybir.ActivationFunctionType.Identity,
                                bias=bt[:], scale=1.0)
                            nc.vector.tensor_tensor(
                                out=st_ap, in0=ps_hi, in1=st_ap,
                                op=mybir.AluOpType.add)
                            if hh == GH - 1:
                                h0 = h - GH + 1
                                nc.gpsimd.dma_start(
                                    out=out[:, :, h0:h0 + GH, :].transpose(
                                        [1, 0, 2, 3]),
                                    in_=stages[r][:],
                                )

            for _ in range(reps):
                emit_pass()

    nc.finalize()
    return nc


def _prep_core(xs, weight, bias):
    """xs: (2,32,224,224) f32 -> per-core input map (bf16 grids/weights)."""
    xb = xs.astype(ml_dtypes.bfloat16)

    def _grid(arr, nblk):
        g = arr.reshape(NB, C, -1, 4, W)[:, :, :nblk].transpose(2, 3, 1, 0, 4)
        o = np.zeros((nblk, 128, NWP), ml_dtypes.bfloat16)
        o[:, :, :NW] = g.reshape(nblk, 128, NW)
        return o

    g0 = _grid(xb, NBLK)
    xpad = np.concatenate(
        [xb[:, :, 2:, :],
         np.zeros((NB, C, 2, W), ml_dtypes.bfloat16)], axis=2)
    g1 = _grid(xpad, NBLK - 1)

    # xg2[b][4c+s, 452*r + col] = xrowpad(strip_row(4b+r))[c, col+s]
    # strip_row(h) = h+4 if h even else h
    rowbuf = np.zeros((C, H, NWP), ml_dtypes.bfloat16)
    rowbuf[:, :, :NW] = xb.transpose(1, 2, 0, 3).reshape(C, H, NW)
    g2 = np.zeros((55, 128, 4 * SW), ml_dtypes.bfloat16)
    for r in range(4):
        hs = np.arange(r, 220, 4)           # output rows with this r
        srows = np.where(hs % 2 == 0, hs + 4, hs)   # strip rows
        for s in range(4):
            # g2[b, 4c+s, 452r + col] = rowbuf[c, srows[b], col+s]
            g2[:, s::4, r * SW:(r + 1) * SW] = \
                rowbuf[:, srows, s:s + SW].transpose(1, 0, 2)
    wb = weight.astype(ml_dtypes.bfloat16).astype(np.float32)

    def _main(kws):
        o = np.zeros((2, 128, len(kws) * 64), np.float32)
        for t in range(2):
            for j in range(4):
                for mi, kw in enumerate(kws):
                    o[t, 32 * j:32 * j + 32, 64 * mi:64 * mi + 64] = \
                        wb[:, :, j + t, kw].T
        return o.astype(ml_dtypes.bfloat16)

    wsm = np.zeros((2, 128, 128), np.float32)
    for i, kh in enumerate((0, 4)):
        for s in range(4):
            wsm[i, s::4, 0:64] = wb[:, :, kh, s].T
        wsm[i, 2::4, 64:128] = wb[:, :, kh, 4].T

    return {
        "xg0": g0,
        "xg1": g1,
        "xg2": g2,
        "w02": _main((0, 2)),
        "w13": _main((1, 3)),
        "w4": _main((4,)),
        "ws": wsm.astype(ml_dtypes.bfloat16),
        "bias": bias.reshape(64, 1).astype(np.float32),
    }


def kernel(x, weight, bias, _profile=False):
    x = np.asarray(x, dtype=np.float32)
    weight = np.asarray(weight, dtype=np.float32)
    bias = np.asarray(bias, dtype=np.float32)

    if "nc" not in _cache:
        _cache["nc"] = _build()
    nc = _cache["nc"]

    in_maps = [
        _prep_core(x[NB * i:NB * i + NB], weight, bias) for i in range(N_CORES)
    ]
    res = run_bass_kernel_spmd(
        nc, in_maps, core_ids=list(range(N_CORES)), trace=_profile)
    out = np.concatenate(
        [np.asarray(r["out"]).astype(np.float32) for r in res.results], axis=0)
    if _profile:
        _cache["last_results"] = res
    return out


if __name__ == "__main__":
    rng = np.random.default_rng(0)
    x = rng.standard_normal((B, C, H, W), dtype=np.float32)
    w = rng.standard_normal((F, C, K, K), dtype=np.float32)
    bv = rng.standard_normal((F,), dtype=np.float32)
    o = kernel(x, w, bv)
    print("output shape:", o.shape, o.dtype)
